# revision 24
# baseline (speedup 1.0000x reference)
"""DTM layer (distance-to-measure) Trainium2 kernel — annulus design.

Math: for each (batch b, grid point n), with squared distances
d2[m] = ||grid_n - x_{b,m}||^2 and wb = 0.3*M, k = ceil(wb):

    dtm = sqrt(F / wb),  F = sum_m min(d2_m, T) - (M - wb)*T

evaluated at T ~= d2_(k) (k-th smallest); F is first-order insensitive
to the error in T (dF/dT = wb - c(T) ~= 0 at T*).

Annulus trick: the grid is host-permuted into 80 compact patches of
128 points (8 x-bands x 10 y-tiles, ~0.25 x 0.2 extent).  For each
(patch, batch) the host computes the exact k-th center distance dk and
patch radius rho; 1-Lipschitz-ness of the k-NN radius bounds every
row's sqrt(T*) within [dk-rho, dk+rho], so points with
d(m,c) < dk-2rho-eps are below T for EVERY row (their contribution is
the closed-form sum n*|g|^2 - 2g.Sx + S|x|^2 via a K=4 matmul) and
points with d(m,c) > dk+2rho+eps contribute exactly T each.  Only the
~1400 annulus points per (patch, batch) are shipped (index-ordered,
padded with far dummies to a per-slot 512-granular width W in
{1536, 2048}, equalized across cores by size-ranked slot assignment)
and scanned on device — 2.5x less than M=4096 — and device T is
clamped into [(dk-rho)^2, (dk+rho)^2] to keep the classification
valid.  Count pass = first 512 annulus cols (index-ordered =>
unbiased), one Newton step with host slope beta, then the final scan.
Numpy sim of this exact pipeline: max rel err ~4.6e-3 (gate 2e-2).

Device mapping (per NeuronCore, 10 patches x 4 batches = 40 pairs):
  - d2 into PSUM by K=12 bf16 matmuls: features (gx, gy, g2, 1) x
    (-2x0, -2x1, 1, |x|^2), each side split hi/lo in bf16, stacked
    [hi_g; hi_g; lo_g] . [hi_x; lo_x; hi_x] -> near-fp32 d2.
  - PSUM ring [128, 1024] fp32 x 4 bufs (8 banks); each tile read by
    exactly ONE engine (two engines on one tile serializes; measured).
  - count: 2 quarter-filled tiles/pair (256 cols each); DVE
    tensor_scalar(is_le, accum), ACT activation(Sign, bias=T0,
    scale=-1, accum).
  - Newton chain (5 ops): T = clamp(C1 - C2*(cD + 0.5*cA), cLo, cHi)
    with per-column host constants.
  - final: 2 full tiles/pair; DVE min-accum, ACT Relu-accum;
    F = nearF + sD - gA + kap*T.
"""

import numpy as np

# ---------------- problem constants (hardcoded per contract) ----------------
B = 4            # batches
M = 4096         # points per batch
N = 10201        # grid points (101 x 101)
G = 101
NCORES = 8
NT = 10          # patches (slots) per core
NTILE = NCORES * NT
NPC = NT * 128   # grid slots per core
WB = 0.3 * M     # 1228.8
KK = int(np.ceil(WB))  # 1229
NSC = NT * B     # 40 state columns (slot, b)
W_CNT = 512      # count-pass subsample width (256 per engine)
EPS = 0.03       # radius slack for bf16 d2 error
DELTA = 64       # half-window for the Newton slope beta

_cache = {}


def _build_nc(reps=1):
    import contextlib
    import concourse.tile as tile
    from concourse import bacc, mybir

    W, OFFS, TOTW = _cache["plan_w"]
    f32 = mybir.dt.float32
    Alu = mybir.AluOpType
    Act = mybir.ActivationFunctionType

    nc = bacc.Bacc("TRN2")
    bf16 = mybir.dt.bfloat16
    gf4 = nc.dram_tensor("gf4", [4, NPC], f32, kind="ExternalInput")
    nearc = nc.dram_tensor("nearc", [4, NSC], f32, kind="ExternalInput")
    cst = nc.dram_tensor("cst", [128, 6 * NSC], f32, kind="ExternalInput")
    gstk = nc.dram_tensor("gstk", [12, NPC], bf16, kind="ExternalInput")
    xann = nc.dram_tensor("xann", [12, TOTW], bf16, kind="ExternalInput")
    out_d = nc.dram_tensor("out", [128, NSC], f32, kind="ExternalOutput")

    with tile.TileContext(nc) as tc:
        with tc.tile_pool(name="sing", bufs=1) as sing:
            # ---- inputs to SBUF ----
            gf = sing.tile([4, NPC], f32)
            ncf = sing.tile([4, NSC], f32)
            cs = sing.tile([128, 6 * NSC], f32)
            gsk = sing.tile([12, NPC], bf16)
            xak = sing.tile([12, TOTW], bf16)
            nc.sync.dma_start(gf[:, :], gf4[:, :])
            nc.sync.dma_start(ncf[:, :], nearc[:, :])
            nc.sync.dma_start(cs[:, :], cst[:, :])
            nc.sync.dma_start(gsk[:, :], gstk[:, :])
            nc.gpsimd.dma_start(xak[:, 0:TOTW // 2], xann[:, 0:TOTW // 2])
            nc.gpsimd.dma_start(xak[:, TOTW // 2:], xann[:, TOTW // 2:])

            # host-constant blocks of cst: [T0, C1, C2, cLo, cHi, kap]
            def cblk(i):
                return cs[:, i * NSC:(i + 1) * NSC]

            # ---- state tiles [128, NSC], col = s*B + b ----
            nearF = sing.tile([128, NSC], f32)
            T = sing.tile([128, NSC], f32)
            cD = sing.tile([128, NSC], f32)   # DVE count partial
            cA = sing.tile([128, NSC], f32)   # ACT sign-sum partial
            t1 = sing.tile([128, NSC], f32)
            t2 = sing.tile([128, NSC], f32)
            sD = sing.tile([128, NSC], f32)   # final DVE min-sum
            gA = sing.tile([128, NSC], f32)   # final ACT relu-sum
            Fv = sing.tile([128, NSC], f32)
            outv = sing.tile([128, NSC], f32)
            scrD = sing.tile([128, 1024], f32)
            scrA = sing.tile([128, 1024], f32)

            def lhsT(s):
                return gsk[0:12, s * 128:(s + 1) * 128]

            # ---- nearF matmul: gf4 rows (gx, gy, g2, 1) x nearc ----
            with tc.tile_pool(name="pmom", bufs=2, space="PSUM") as pmom:
                for s in range(NT):
                    psm = pmom.tile([128, B], f32, tag="mom")
                    nc.tensor.matmul(
                        psm[:, :],
                        gf[0:4, s * 128:(s + 1) * 128],
                        ncf[0:4, s * B:(s + 1) * B],
                        start=True, stop=True,
                    )
                    nc.vector.tensor_copy(nearF[:, s * B:(s + 1) * B], psm[:, :])

            # device-side repetition loop for timing (reps=1: no loop)
            rep_ctx = tc.For_i(0, reps, 1) if reps > 1 else contextlib.nullcontext()
            with rep_ctx:
              nc.vector.tensor_copy(T[:, :], cblk(0)[:, :])   # T := T0
              with tc.tile_pool(name="pd2", bufs=4, space="PSUM") as pd2:
                  def gen_tile(s, o0, width):
                      """Matmuls producing d2[128 x width] in a PSUM tile
                      from xann flat cols [o0 : o0+width]."""
                      ps = pd2.tile([128, 1024], f32, tag="q")
                      j = 0
                      while width > 0:
                          wj = min(512, width)
                          nc.tensor.matmul(
                              ps[:, j * 512:j * 512 + wj],
                              lhsT(s), xak[0:12, o0:o0 + wj],
                              start=True, stop=True,
                          )
                          o0 += wj
                          width -= wj
                          j += 1
                      return ps

                  # ---- count pass at T0 over first W_CNT annulus cols ----
                  WH = W_CNT // 2
                  for s in range(NT):
                      for b in range(B):
                          col = s * B + b
                          o0 = int(OFFS[s, b])
                          ps0 = gen_tile(s, o0, WH)
                          nc.vector.tensor_scalar(
                              scrD[:, 0:WH], ps0[:, 0:WH],
                              T[:, col:col + 1], None,
                              op0=Alu.is_le, op1=Alu.add,
                              accum_out=cD[:, col:col + 1])
                          ps1 = gen_tile(s, o0 + WH, WH)
                          nc.scalar.activation(
                              scrA[:, 0:WH], ps1[:, 0:WH], Act.Sign,
                              bias=T[:, col:col + 1], scale=-1.0,
                              accum_out=cA[:, col:col + 1])

                  # ---- Newton: T = clamp(C1 - C2*(cD + 0.5 cA)) ----
                  nc.vector.scalar_tensor_tensor(
                      t1[:, :], cA[:, :], 0.5, cD[:, :],
                      op0=Alu.mult, op1=Alu.add)
                  nc.vector.tensor_mul(t2[:, :], t1[:, :], cblk(2)[:, :])
                  nc.vector.tensor_sub(T[:, :], cblk(1)[:, :], t2[:, :])
                  nc.vector.tensor_max(T[:, :], T[:, :], cblk(3)[:, :])
                  nc.vector.tensor_tensor(T[:, :], T[:, :], cblk(4)[:, :],
                                          op=Alu.min)

                  # ---- final pass over the annulus ----
                  for s in range(NT):
                      for b in range(B):
                          col = s * B + b
                          o0 = int(OFFS[s, b])
                          wd = int(W[s, b]) // 2
                          ps0 = gen_tile(s, o0, wd)
                          nc.vector.tensor_scalar(
                              scrD[:, 0:wd], ps0[:, 0:wd],
                              T[:, col:col + 1], None,
                              op0=Alu.min, op1=Alu.add,
                              accum_out=sD[:, col:col + 1])
                          ps1 = gen_tile(s, o0 + wd, wd)
                          nc.scalar.activation(
                              scrA[:, 0:wd], ps1[:, 0:wd], Act.Relu,
                              bias=T[:, col:col + 1], scale=-1.0,
                              accum_out=gA[:, col:col + 1])

              # F = nearF + sD - gA + kap*T ;  out = sqrt(F / WB)
              nc.vector.tensor_sub(Fv[:, :], sD[:, :], gA[:, :])
              nc.vector.tensor_mul(t2[:, :], T[:, :], cblk(5)[:, :])
              nc.vector.tensor_add(Fv[:, :], Fv[:, :], t2[:, :])
              nc.vector.tensor_add(Fv[:, :], Fv[:, :], nearF[:, :])
              nc.vector.tensor_scalar_max(Fv[:, :], Fv[:, :], 0.0)
              nc.scalar.activation(outv[:, :], Fv[:, :], Act.Sqrt, scale=1.0 / WB)
              nc.sync.dma_start(out_d[:, :], outv[:, :])

    nc.finalize()
    return nc


def _split_hl(v32):
    import ml_dtypes
    bf = ml_dtypes.bfloat16
    v = np.asarray(v32, np.float64)
    hi = v.astype(bf)
    lo = (v - hi.astype(np.float64)).astype(bf)
    return hi, lo


def _plan(x, grid):
    """Host geometry: patches, classification, constants, gathers."""
    x = np.asarray(x, np.float64)
    grid = np.asarray(grid, np.float64)
    NTOT = NTILE * 128
    idx_all = np.arange(N, dtype=np.int64)
    pads = np.full(NTOT - N, N - 1, np.int64)
    pool = np.concatenate([idx_all, pads])
    xs_c = np.tile(np.linspace(-1, 1, G), G)      # x coord of grid idx
    ys_c = np.repeat(np.linspace(-1, 1, G), G)    # y coord

    def split(ids, coord, parts):
        order = np.argsort(coord[ids], kind="stable")
        ids = ids[order]
        n = len(ids) // parts
        return [ids[i * n:(i + 1) * n] for i in range(parts)]

    tiles = []
    for band in split(pool, xs_c, 8):
        tiles.extend(split(band, ys_c, 10))

    # per-(tile, b) geometry
    per_core = {c: {} for c in range(NCORES)}
    sizes = []
    geo = []
    for t, ids in enumerate(tiles):
        pts = grid[ids]
        c0 = pts.mean(0)
        rho = np.sqrt(((pts - c0) ** 2).sum(-1)).max()
        ent = {"ids": ids, "pts": pts, "rho": rho, "b": []}
        mx = 0
        for b in range(B):
            d = np.sqrt(((x[b] - c0) ** 2).sum(-1))
            so = np.argsort(d)
            ds = d[so]
            dk = ds[KK - 1]
            beta = (ds[KK - 1 + DELTA] ** 2 - ds[KK - 1 - DELTA] ** 2) \
                / (2 * DELTA)
            lo_r = dk - 2 * rho - EPS
            hi_r = dk + 2 * rho + EPS
            near = so[ds < lo_r]
            ann = np.sort(so[(ds >= lo_r) & (ds <= hi_r)])
            ent["b"].append({
                "dk": dk, "beta": beta, "near": near, "ann": ann,
                "clamp_lo": max((dk - rho) ** 2, 0.0),
                "clamp_hi": (dk + rho) ** 2,
            })
            mx = max(mx, len(ann))
        sizes.append(mx)
        geo.append(ent)

    # slot assignment: rank by size desc -> core r%8, slot r//8
    order = np.argsort(np.array(sizes) * -1, kind="stable")
    for r, t in enumerate(order):
        per_core[r % NCORES][r // NCORES] = t
    # 512-granular per-(slot, b) widths = max over cores
    W = np.zeros((NT, B), np.int64)
    for c in range(NCORES):
        for s in range(NT):
            e = geo[per_core[c][s]]
            for b in range(B):
                W[s, b] = max(W[s, b], len(e["b"][b]["ann"]))
    W = ((W + 511) // 512) * 512
    offs = np.zeros((NT, B), np.int64)
    acc = 0
    for s in range(NT):
        for b in range(B):
            offs[s, b] = acc
            acc += W[s, b]
    return geo, per_core, W, offs, acc


def _in_maps(x, grid):
    x64 = np.asarray(x, np.float64)
    grid64 = np.asarray(grid, np.float64)
    geo, per_core, W, offs, totw = _plan(x64, grid64)
    _cache["plan"] = (geo, per_core)
    _cache["plan_w"] = (W, offs, totw)

    maps = []
    for c in range(NCORES):
        totw_c = totw
        gf4 = np.zeros((4, NPC), np.float32)
        nearc = np.zeros((4, NSC), np.float32)
        cst = np.zeros((128, 6 * NSC), np.float32)
        gstk = np.zeros((12, NPC), np.float32)
        xann = np.zeros((12, totw_c), np.float32)
        for s in range(NT):
            t = per_core[c][s]
            e = geo[t]
            pts = e["pts"]
            gx, gy = pts[:, 0], pts[:, 1]
            g2 = gx * gx + gy * gy
            gfeat = np.stack([gx, gy, g2, np.ones_like(gx)], 0)
            gf4[:, s * 128:(s + 1) * 128] = gfeat
            g_hi, g_lo = _split_hl(gfeat)
            gstk[:, s * 128:(s + 1) * 128] = np.concatenate(
                [g_hi, g_hi, g_lo], 0)
            for b in range(B):
                eb = e["b"][b]
                col = s * B + b
                ann = eb["ann"]
                n_ann = len(ann)
                n_near = len(eb["near"])
                w_sb = int(W[s, b])
                o_sb = int(offs[s, b])
                xnear = x64[b][eb["near"]]
                nearc[:, col] = [-2 * xnear[:, 0].sum(),
                                 -2 * xnear[:, 1].sum(),
                                 float(n_near),
                                 (xnear ** 2).sum()]
                # annulus features, padded with far dummies
                x0 = np.concatenate([x64[b][ann, 0],
                                     np.full(w_sb - n_ann, 200.0)])
                x1 = np.concatenate([x64[b][ann, 1],
                                     np.zeros(w_sb - n_ann)])
                xf = np.stack([-2 * x0, -2 * x1, np.ones_like(x0),
                               x0 * x0 + x1 * x1], 0)
                x_hi, x_lo = _split_hl(xf)
                xann[:, o_sb:o_sb + w_sb] = np.concatenate(
                    [x_hi, x_lo, x_hi], 0)
                # constants: T0, C1, C2, cLo, cHi, kap
                T0 = eb["dk"] ** 2
                scale = n_ann / min(W_CNT, n_ann)
                C1 = T0 + (WB - n_near - (W_CNT / 4) * scale) * eb["beta"]
                C2 = eb["beta"] * scale
                n_far = M - n_near - n_ann
                # + w/2: ACT-half min-sum is (w/2)*T - gA, the (w/2)*T
                # part folds in here
                kap = -(w_sb - n_ann) + n_far - (M - WB) + w_sb // 2
                cst[:, 0 * NSC + col] = T0
                cst[:, 1 * NSC + col] = C1
                cst[:, 2 * NSC + col] = C2
                cst[:, 3 * NSC + col] = eb["clamp_lo"]
                cst[:, 4 * NSC + col] = eb["clamp_hi"]
                cst[:, 5 * NSC + col] = kap
        import ml_dtypes
        maps.append({
            "gf4": np.ascontiguousarray(gf4),
            "nearc": np.ascontiguousarray(nearc),
            "cst": np.ascontiguousarray(cst),
            "gstk": np.ascontiguousarray(gstk.astype(ml_dtypes.bfloat16)),
            "xann": np.ascontiguousarray(xann.astype(ml_dtypes.bfloat16)),
        })
    return maps


def _get_nc():
    W, offs, totw = _cache["plan_w"]
    sig = (totw, W.tobytes(), offs.tobytes())
    if _cache.get("nc_sig") != sig:
        _cache["nc"] = _build_nc()
        _cache["nc_sig"] = sig
    return _cache["nc"]


def kernel(x, grid, _trace=False):
    from concourse.bass_utils import run_bass_kernel_spmd

    in_maps = _in_maps(x, grid)
    nc = _get_nc()
    res = run_bass_kernel_spmd(nc, in_maps, core_ids=list(range(NCORES)),
                               trace=_trace)
    _cache["last_result"] = res
    geo, per_core = _cache["plan"]
    full = np.zeros((B, N), np.float32)
    for c in range(NCORES):
        o = res.results[c]["out"].reshape(128, NT, B)
        for s in range(NT):
            ids = geo[per_core[c][s]]["ids"]
            for b in range(B):
                full[b][ids] = o[:, s, b]
    return full


# revision 25
# speedup vs baseline: 1.0540x; 1.0540x over previous
"""DTM layer (distance-to-measure) Trainium2 kernel — annulus design.

Math: for each (batch b, grid point n), with squared distances
d2[m] = ||grid_n - x_{b,m}||^2 and wb = 0.3*M, k = ceil(wb):

    dtm = sqrt(F / wb),  F = sum_m min(d2_m, T) - (M - wb)*T

evaluated at T ~= d2_(k) (k-th smallest); F is first-order insensitive
to the error in T (dF/dT = wb - c(T) ~= 0 at T*).

Annulus trick: the grid is host-permuted into 80 compact patches of
128 points (8 x-bands x 10 y-tiles, ~0.25 x 0.2 extent).  For each
(patch, batch) the host computes the exact k-th center distance dk and
patch radius rho; 1-Lipschitz-ness of the k-NN radius bounds every
row's sqrt(T*) within [dk-rho, dk+rho], so points with
d(m,c) < dk-2rho-eps are below T for EVERY row (their contribution is
the closed-form sum n*|g|^2 - 2g.Sx + S|x|^2 via a K=4 matmul) and
points with d(m,c) > dk+2rho+eps contribute exactly T each.  Only the
~1400 annulus points per (patch, batch) are shipped (index-ordered,
padded with far dummies to a per-slot 512-granular width W in
{1536, 2048}, equalized across cores by size-ranked slot assignment)
and scanned on device — 2.5x less than M=4096 — and device T is
clamped into [(dk-rho)^2, (dk+rho)^2] to keep the classification
valid.  Count pass = first 512 annulus cols (index-ordered =>
unbiased), one Newton step with host slope beta, then the final scan.
Numpy sim of this exact pipeline: max rel err ~4.6e-3 (gate 2e-2).

Device mapping (per NeuronCore, 10 patches x 4 batches = 40 pairs):
  - d2 into PSUM by K=12 bf16 matmuls: features (gx, gy, g2, 1) x
    (-2x0, -2x1, 1, |x|^2), each side split hi/lo in bf16, stacked
    [hi_g; hi_g; lo_g] . [hi_x; lo_x; hi_x] -> near-fp32 d2.
  - PSUM ring [128, 1024] fp32 x 4 bufs (8 banks); each tile read by
    exactly ONE engine (two engines on one tile serializes; measured).
  - count: 2 quarter-filled tiles/pair (256 cols each); DVE
    tensor_scalar(is_le, accum), ACT activation(Sign, bias=T0,
    scale=-1, accum).
  - Newton chain (5 ops): T = clamp(C1 - C2*(cD + 0.5*cA), cLo, cHi)
    with per-column host constants.
  - final: 2 full tiles/pair; DVE min-accum, ACT Relu-accum;
    F = nearF + sD - gA + kap*T.
"""

import numpy as np

# ---------------- problem constants (hardcoded per contract) ----------------
B = 4            # batches
M = 4096         # points per batch
N = 10201        # grid points (101 x 101)
G = 101
NCORES = 8
NT = 10          # patches (slots) per core
NTILE = NCORES * NT
NPC = NT * 128   # grid slots per core
WB = 0.3 * M     # 1228.8
KK = int(np.ceil(WB))  # 1229
NSC = NT * B     # 40 state columns (slot, b)
W_CNT = 512      # count-pass subsample width (256 per engine)
EPS = 0.0        # clamp bracket alone guarantees validity
DELTA = 64       # half-window for the Newton slope beta

_cache = {}


def _build_nc(reps=1):
    import contextlib
    import concourse.tile as tile
    from concourse import bacc, mybir

    W, OFFS, TOTW = _cache["plan_w"]
    f32 = mybir.dt.float32
    Alu = mybir.AluOpType
    Act = mybir.ActivationFunctionType

    nc = bacc.Bacc("TRN2")
    bf16 = mybir.dt.bfloat16
    gf4 = nc.dram_tensor("gf4", [4, NPC], f32, kind="ExternalInput")
    nearc = nc.dram_tensor("nearc", [4, NSC], f32, kind="ExternalInput")
    cst = nc.dram_tensor("cst", [128, 6 * NSC], f32, kind="ExternalInput")
    gstk = nc.dram_tensor("gstk", [12, NPC], bf16, kind="ExternalInput")
    xann = nc.dram_tensor("xann", [12, TOTW], bf16, kind="ExternalInput")
    out_d = nc.dram_tensor("out", [128, NSC], f32, kind="ExternalOutput")

    with tile.TileContext(nc) as tc:
        with tc.tile_pool(name="sing", bufs=1) as sing:
            # ---- inputs to SBUF ----
            gf = sing.tile([4, NPC], f32)
            ncf = sing.tile([4, NSC], f32)
            cs = sing.tile([128, 6 * NSC], f32)
            gsk = sing.tile([12, NPC], bf16)
            xak = sing.tile([12, TOTW], bf16)
            nc.sync.dma_start(gf[:, :], gf4[:, :])
            nc.sync.dma_start(ncf[:, :], nearc[:, :])
            nc.sync.dma_start(cs[:, :], cst[:, :])
            nc.sync.dma_start(gsk[:, :], gstk[:, :])
            nc.gpsimd.dma_start(xak[:, 0:TOTW // 2], xann[:, 0:TOTW // 2])
            nc.gpsimd.dma_start(xak[:, TOTW // 2:], xann[:, TOTW // 2:])

            # host-constant blocks of cst: [T0, C1, C2, cLo, cHi, kap]
            def cblk(i):
                return cs[:, i * NSC:(i + 1) * NSC]

            # ---- state tiles [128, NSC], col = s*B + b ----
            nearF = sing.tile([128, NSC], f32)
            T = sing.tile([128, NSC], f32)
            cD = sing.tile([128, NSC], f32)   # DVE count partial
            cA = sing.tile([128, NSC], f32)   # ACT sign-sum partial
            t1 = sing.tile([128, NSC], f32)
            t2 = sing.tile([128, NSC], f32)
            sD = sing.tile([128, NSC], f32)   # final DVE min-sum
            gA = sing.tile([128, NSC], f32)   # final ACT relu-sum
            Fv = sing.tile([128, NSC], f32)
            outv = sing.tile([128, NSC], f32)
            scrD = sing.tile([128, 1024], f32)
            scrA = sing.tile([128, 1024], f32)

            def lhsT(s):
                return gsk[0:12, s * 128:(s + 1) * 128]

            # ---- nearF matmul: gf4 rows (gx, gy, g2, 1) x nearc ----
            with tc.tile_pool(name="pmom", bufs=2, space="PSUM") as pmom:
                for s in range(NT):
                    psm = pmom.tile([128, B], f32, tag="mom")
                    nc.tensor.matmul(
                        psm[:, :],
                        gf[0:4, s * 128:(s + 1) * 128],
                        ncf[0:4, s * B:(s + 1) * B],
                        start=True, stop=True,
                    )
                    nc.vector.tensor_copy(nearF[:, s * B:(s + 1) * B], psm[:, :])

            # device-side repetition loop for timing (reps=1: no loop)
            rep_ctx = tc.For_i(0, reps, 1) if reps > 1 else contextlib.nullcontext()
            with rep_ctx:
              nc.vector.tensor_copy(T[:, :], cblk(0)[:, :])   # T := T0
              with tc.tile_pool(name="pd2", bufs=4, space="PSUM") as pd2:
                  def gen_tile(s, o0, width):
                      """Matmuls producing d2[128 x width] in a PSUM tile
                      from xann flat cols [o0 : o0+width]."""
                      ps = pd2.tile([128, 1024], f32, tag="q")
                      j = 0
                      while width > 0:
                          wj = min(512, width)
                          nc.tensor.matmul(
                              ps[:, j * 512:j * 512 + wj],
                              lhsT(s), xak[0:12, o0:o0 + wj],
                              start=True, stop=True,
                          )
                          o0 += wj
                          width -= wj
                          j += 1
                      return ps

                  # ---- count pass at T0 over first W_CNT annulus cols ----
                  WH = W_CNT // 2
                  for s in range(NT):
                      for b in range(B):
                          col = s * B + b
                          o0 = int(OFFS[s, b])
                          ps0 = gen_tile(s, o0, WH)
                          nc.vector.tensor_scalar(
                              scrD[:, 0:WH], ps0[:, 0:WH],
                              T[:, col:col + 1], None,
                              op0=Alu.is_le, op1=Alu.add,
                              accum_out=cD[:, col:col + 1])
                          ps1 = gen_tile(s, o0 + WH, WH)
                          nc.scalar.activation(
                              scrA[:, 0:WH], ps1[:, 0:WH], Act.Sign,
                              bias=T[:, col:col + 1], scale=-1.0,
                              accum_out=cA[:, col:col + 1])

                  # ---- Newton: T = clamp(C1 - C2*(cD + 0.5 cA)) ----
                  nc.vector.scalar_tensor_tensor(
                      t1[:, :], cA[:, :], 0.5, cD[:, :],
                      op0=Alu.mult, op1=Alu.add)
                  nc.vector.tensor_mul(t2[:, :], t1[:, :], cblk(2)[:, :])
                  nc.vector.tensor_sub(T[:, :], cblk(1)[:, :], t2[:, :])
                  nc.vector.tensor_max(T[:, :], T[:, :], cblk(3)[:, :])
                  nc.vector.tensor_tensor(T[:, :], T[:, :], cblk(4)[:, :],
                                          op=Alu.min)

                  # ---- final pass over the annulus ----
                  for s in range(NT):
                      for b in range(B):
                          col = s * B + b
                          o0 = int(OFFS[s, b])
                          wd = int(W[s, b]) // 2
                          ps0 = gen_tile(s, o0, wd)
                          nc.vector.tensor_scalar(
                              scrD[:, 0:wd], ps0[:, 0:wd],
                              T[:, col:col + 1], None,
                              op0=Alu.min, op1=Alu.add,
                              accum_out=sD[:, col:col + 1])
                          ps1 = gen_tile(s, o0 + wd, wd)
                          nc.scalar.activation(
                              scrA[:, 0:wd], ps1[:, 0:wd], Act.Relu,
                              bias=T[:, col:col + 1], scale=-1.0,
                              accum_out=gA[:, col:col + 1])

              # F = nearF + sD - gA + kap*T ;  out = sqrt(F / WB)
              nc.vector.tensor_sub(Fv[:, :], sD[:, :], gA[:, :])
              nc.vector.tensor_mul(t2[:, :], T[:, :], cblk(5)[:, :])
              nc.vector.tensor_add(Fv[:, :], Fv[:, :], t2[:, :])
              nc.vector.tensor_add(Fv[:, :], Fv[:, :], nearF[:, :])
              nc.vector.tensor_scalar_max(Fv[:, :], Fv[:, :], 0.0)
              nc.scalar.activation(outv[:, :], Fv[:, :], Act.Sqrt, scale=1.0 / WB)
              nc.sync.dma_start(out_d[:, :], outv[:, :])

    nc.finalize()
    return nc


def _split_hl(v32):
    import ml_dtypes
    bf = ml_dtypes.bfloat16
    v = np.asarray(v32, np.float64)
    hi = v.astype(bf)
    lo = (v - hi.astype(np.float64)).astype(bf)
    return hi, lo


def _plan(x, grid):
    """Host geometry: patches, classification, constants, gathers."""
    x = np.asarray(x, np.float64)
    grid = np.asarray(grid, np.float64)
    NTOT = NTILE * 128
    idx_all = np.arange(N, dtype=np.int64)
    pads = np.full(NTOT - N, N - 1, np.int64)
    pool = np.concatenate([idx_all, pads])
    xs_c = np.tile(np.linspace(-1, 1, G), G)      # x coord of grid idx
    ys_c = np.repeat(np.linspace(-1, 1, G), G)    # y coord

    def split(ids, coord, parts):
        order = np.argsort(coord[ids], kind="stable")
        ids = ids[order]
        n = len(ids) // parts
        return [ids[i * n:(i + 1) * n] for i in range(parts)]

    tiles = []
    for band in split(pool, xs_c, 8):
        tiles.extend(split(band, ys_c, 10))

    # per-(tile, b) geometry
    per_core = {c: {} for c in range(NCORES)}
    sizes = []
    geo = []
    for t, ids in enumerate(tiles):
        pts = grid[ids]
        c0 = pts.mean(0)
        rho = np.sqrt(((pts - c0) ** 2).sum(-1)).max()
        ent = {"ids": ids, "pts": pts, "rho": rho, "b": []}
        mx = 0
        for b in range(B):
            d = np.sqrt(((x[b] - c0) ** 2).sum(-1))
            so = np.argsort(d)
            ds = d[so]
            dk = ds[KK - 1]
            beta = (ds[KK - 1 + DELTA] ** 2 - ds[KK - 1 - DELTA] ** 2) \
                / (2 * DELTA)
            lo_r = dk - 2 * rho - EPS
            hi_r = dk + 2 * rho + EPS
            near = so[ds < lo_r]
            ann = np.sort(so[(ds >= lo_r) & (ds <= hi_r)])
            ent["b"].append({
                "dk": dk, "beta": beta, "near": near, "ann": ann,
                "clamp_lo": max((dk - rho) ** 2, 0.0),
                "clamp_hi": (dk + rho) ** 2,
            })
            mx = max(mx, len(ann))
        sizes.append(mx)
        geo.append(ent)

    # slot assignment: rank by size desc -> core r%8, slot r//8
    order = np.argsort(np.array(sizes) * -1, kind="stable")
    for r, t in enumerate(order):
        per_core[r % NCORES][r // NCORES] = t
    # 512-granular per-(slot, b) widths = max over cores
    W = np.zeros((NT, B), np.int64)
    for c in range(NCORES):
        for s in range(NT):
            e = geo[per_core[c][s]]
            for b in range(B):
                W[s, b] = max(W[s, b], len(e["b"][b]["ann"]))
    W = ((W + 255) // 256) * 256
    assert W.min() >= 512
    offs = np.zeros((NT, B), np.int64)
    acc = 0
    for s in range(NT):
        for b in range(B):
            offs[s, b] = acc
            acc += W[s, b]
    return geo, per_core, W, offs, acc


def _in_maps(x, grid):
    x64 = np.asarray(x, np.float64)
    grid64 = np.asarray(grid, np.float64)
    geo, per_core, W, offs, totw = _plan(x64, grid64)
    _cache["plan"] = (geo, per_core)
    _cache["plan_w"] = (W, offs, totw)

    maps = []
    for c in range(NCORES):
        totw_c = totw
        gf4 = np.zeros((4, NPC), np.float32)
        nearc = np.zeros((4, NSC), np.float32)
        cst = np.zeros((128, 6 * NSC), np.float32)
        gstk = np.zeros((12, NPC), np.float32)
        xann = np.zeros((12, totw_c), np.float32)
        for s in range(NT):
            t = per_core[c][s]
            e = geo[t]
            pts = e["pts"]
            gx, gy = pts[:, 0], pts[:, 1]
            g2 = gx * gx + gy * gy
            gfeat = np.stack([gx, gy, g2, np.ones_like(gx)], 0)
            gf4[:, s * 128:(s + 1) * 128] = gfeat
            g_hi, g_lo = _split_hl(gfeat)
            gstk[:, s * 128:(s + 1) * 128] = np.concatenate(
                [g_hi, g_hi, g_lo], 0)
            for b in range(B):
                eb = e["b"][b]
                col = s * B + b
                ann = eb["ann"]
                n_ann = len(ann)
                n_near = len(eb["near"])
                w_sb = int(W[s, b])
                o_sb = int(offs[s, b])
                xnear = x64[b][eb["near"]]
                nearc[:, col] = [-2 * xnear[:, 0].sum(),
                                 -2 * xnear[:, 1].sum(),
                                 float(n_near),
                                 (xnear ** 2).sum()]
                # annulus features, padded with far dummies
                x0 = np.concatenate([x64[b][ann, 0],
                                     np.full(w_sb - n_ann, 200.0)])
                x1 = np.concatenate([x64[b][ann, 1],
                                     np.zeros(w_sb - n_ann)])
                xf = np.stack([-2 * x0, -2 * x1, np.ones_like(x0),
                               x0 * x0 + x1 * x1], 0)
                x_hi, x_lo = _split_hl(xf)
                xann[:, o_sb:o_sb + w_sb] = np.concatenate(
                    [x_hi, x_lo, x_hi], 0)
                # constants: T0, C1, C2, cLo, cHi, kap
                T0 = eb["dk"] ** 2
                scale = n_ann / min(W_CNT, n_ann)
                C1 = T0 + (WB - n_near - (W_CNT / 4) * scale) * eb["beta"]
                C2 = eb["beta"] * scale
                n_far = M - n_near - n_ann
                # + w/2: ACT-half min-sum is (w/2)*T - gA, the (w/2)*T
                # part folds in here
                kap = -(w_sb - n_ann) + n_far - (M - WB) + w_sb // 2
                cst[:, 0 * NSC + col] = T0
                cst[:, 1 * NSC + col] = C1
                cst[:, 2 * NSC + col] = C2
                cst[:, 3 * NSC + col] = eb["clamp_lo"]
                cst[:, 4 * NSC + col] = eb["clamp_hi"]
                cst[:, 5 * NSC + col] = kap
        import ml_dtypes
        maps.append({
            "gf4": np.ascontiguousarray(gf4),
            "nearc": np.ascontiguousarray(nearc),
            "cst": np.ascontiguousarray(cst),
            "gstk": np.ascontiguousarray(gstk.astype(ml_dtypes.bfloat16)),
            "xann": np.ascontiguousarray(xann.astype(ml_dtypes.bfloat16)),
        })
    return maps


def _get_nc():
    W, offs, totw = _cache["plan_w"]
    sig = (totw, W.tobytes(), offs.tobytes())
    if _cache.get("nc_sig") != sig:
        _cache["nc"] = _build_nc()
        _cache["nc_sig"] = sig
    return _cache["nc"]


def kernel(x, grid, _trace=False):
    from concourse.bass_utils import run_bass_kernel_spmd

    in_maps = _in_maps(x, grid)
    nc = _get_nc()
    res = run_bass_kernel_spmd(nc, in_maps, core_ids=list(range(NCORES)),
                               trace=_trace)
    _cache["last_result"] = res
    geo, per_core = _cache["plan"]
    full = np.zeros((B, N), np.float32)
    for c in range(NCORES):
        o = res.results[c]["out"].reshape(128, NT, B)
        for s in range(NT):
            ids = geo[per_core[c][s]]["ids"]
            for b in range(B):
                full[b][ids] = o[:, s, b]
    return full
